# revision 1
# baseline (speedup 1.0000x reference)
"""Trainium2 Bass kernel for an 8-branch MLP block with layernorm + gelu + skip.

Reference computation (per branch n of 8, batch B=16384, vocab D=256, ffn E=1024):
    h   = gelu_exact(x[:, n, :] @ U1[n] + b1[n])          # (B, E)
    y   = h @ U2[n] + b2[n]                               # (B, D)
    z   = layernorm(y) * ln_w + ln_b
    out[:, n, :] = x[:, n, :] + gelu_exact(z)
Output reshaped to (B, 1, 8*D).

Sharding: expert-parallel - branch n on NeuronCore n (8 cores).

Structure: 1024-row superblocks. GEMM1 accumulates one ffn-chunk (ec) over the
whole superblock into a 2-bank PSUM tile so gelu1 runs as one N=1024 ACT op
per ec (the ACT fixed cost ~224cyc/op makes 8x1024 much cheaper than 16x512).
GEMM2 for superblock i-1 is interleaved into superblock i's ec loop, one
(half, bc) accumulation group per ec step. The layernorm scale/shift is fused
into the gelu2 activation (bias=-mu*rstd, scale=rstd); skip-add runs on the
otherwise-idle Pool engine.
"""

import numpy as np

BATCH, BRANCH, VOCAB, FFN = 16384, 8, 256, 1024
LN_EPS = 1e-5
SBLK = 1024  # batch rows per superblock
NSB = BATCH // SBLK
NBC = 4  # 128-row chunks per half-superblock
NKC = VOCAB // 128  # contraction chunks for GEMM1
NEC = FFN // 128  # e-chunks (GEMM1 output tiles / GEMM2 contraction)

_CACHE = {}
LAST_EXEC_NS = None


def _build(general_ln: bool, reps: int = 1, g1bf: bool = True, g2bf: bool = True,
           xbbf: bool = False, obf: bool = False, zbat: bool = False,
           q4: bool = False, xq: int = 3):
    import concourse.bacc as bacc
    import concourse.tile as tile
    import concourse.mybir as mybir

    f32 = mybir.dt.float32
    bf16 = mybir.dt.bfloat16
    g1dt = bf16 if g1bf else mybir.dt.float32r
    g2dt = bf16 if g2bf else mybir.dt.float32r
    Act = mybir.ActivationFunctionType

    nc = bacc.Bacc(None, target_bir_lowering=False)

    xt = nc.dram_tensor("xt", [VOCAB, BATCH], g1dt, kind="ExternalInput")
    xb = nc.dram_tensor("xb", [BATCH, VOCAB], bf16 if xbbf else f32, kind="ExternalInput")
    u1 = nc.dram_tensor("u1", [VOCAB, FFN], g1dt, kind="ExternalInput")
    u2 = nc.dram_tensor("u2", [FFN, VOCAB], g2dt, kind="ExternalInput")
    b1r = nc.dram_tensor("b1r", [128, NEC], f32, kind="ExternalInput")
    b2bc = nc.dram_tensor("b2bc", [128, NBC, VOCAB], f32, kind="ExternalInput")
    if general_ln:
        lnwbc = nc.dram_tensor("lnwbc", [128, NBC, VOCAB], f32, kind="ExternalInput")
        lnbbc = nc.dram_tensor("lnbbc", [128, NBC, VOCAB], f32, kind="ExternalInput")
    out = nc.dram_tensor("out", [BATCH, VOCAB], bf16 if obf else f32, kind="ExternalOutput")

    with tile.TileContext(nc) as tc:
        with (
            tc.tile_pool(name="singles", bufs=1) as singles,
            tc.tile_pool(name="xtp", bufs=xq) as xtp,
            tc.tile_pool(name="xbp", bufs=3) as xbp,
            tc.tile_pool(name="hp", bufs=2) as hp,
            tc.tile_pool(name="yp", bufs=4) as yp,
            tc.tile_pool(name="sp", bufs=8) as sp,
            tc.tile_pool(name="op", bufs=4) as op,
            tc.tile_pool(name="phq", bufs=3 if q4 else 2, space="PSUM") as phq,
            tc.tile_pool(name="pyq", bufs=2, space="PSUM") as pyq,
        ):
            # --- resident weights / constants. Ordered so the first GEMM1
            # matmul (needs u1[kc=0] + xt0[kc=0]) can start as early as
            # possible ---
            u1_t = singles.tile([128, NKC, FFN], g1dt)
            xt0_t = xtp.tile([128, NKC, SBLK], g1dt, tag="xt")
            u1_view = u1.rearrange("(c p) e -> p c e", p=128)
            xt0_view = xt[:, 0:SBLK].rearrange("(c p) m -> p c m", p=128)
            nc.sync.dma_start(u1_t[:, 0, 0:128], u1_view[:, 0, 0:128])
            nc.sync.dma_start(xt0_t[:, 0, :], xt0_view[:, 0, :])
            nc.sync.dma_start(u1_t[:, 0, 128:FFN], u1_view[:, 0, 128:FFN])
            nc.sync.dma_start(xt0_t[:, 1, :], xt0_view[:, 1, :])
            nc.sync.dma_start(u1_t[:, 1, :], u1_view[:, 1, :])
            u2_t = singles.tile([128, NEC, VOCAB], g2dt)
            u2_view = u2.rearrange("(c p) d -> p c d", p=128)
            nc.sync.dma_start(u2_t[:, 0:2, :], u2_view[:, 0:2, :])
            b1_t = singles.tile([128, NEC], f32)
            nc.sync.dma_start(b1_t[:], b1r[:])
            nc.sync.dma_start(u2_t[:, 2:NEC, :], u2_view[:, 2:NEC, :])

            def prefetch_xt(it):
                i = it % NSB
                t = xtp.tile([128, NKC, SBLK], g1dt, tag="xt")
                nc.sync.dma_start(
                    t[:],
                    xt[:, i * SBLK : (i + 1) * SBLK].rearrange(
                        "(c p) m -> p c m", p=128
                    ),
                )
                return t

            xt_queue = [xt0_t]
            for j in range(1, min(xq, NSB * reps)):
                xt_queue.append(prefetch_xt(j))

            b2_t = singles.tile([128, NBC, VOCAB], f32)
            if general_ln:
                lnw_t = singles.tile([128, NBC, VOCAB], f32)
                nc.sync.dma_start(lnw_t[:], lnwbc[:])
                lnb_t = singles.tile([128, NBC, VOCAB], f32)
                nc.sync.dma_start(lnb_t[:], lnbbc[:])
            magic_t = singles.tile([128, NBC], mybir.dt.int32)
            nc.vector.memset(magic_t[:], 0x5F3759DF)
            # dummy activation: pull the Gelu LUT load into the startup DMA
            # window instead of stalling the first real gelu1
            warm_t = singles.tile([128, 1], f32)
            nc.vector.memset(warm_t[:], 0.0)
            nc.scalar.activation(warm_t[:], warm_t[:], Act.Gelu)
            # dummy matmuls: spend the HAM clock-gate warmup (~3.4us of PE
            # busy before 2.4GHz) inside the startup DMA window on zeroed data
            warm_w = singles.tile([128, 128], g1dt)
            nc.vector.memset(warm_w[:] if g1bf else warm_w[:].bitcast(f32), 0.0)
            warm_r = singles.tile([128, VOCAB], g1dt)
            nc.vector.memset(warm_r[:] if g1bf else warm_r[:].bitcast(f32), 0.0)
            warm_ps = phq.tile([128, SBLK], f32, tag="ph")
            for _ in range(16):
                nc.tensor.matmul(
                    warm_ps[:, 0:VOCAB], warm_w[:], warm_r[:], start=True, stop=True
                )

            # --- epilogue, in two parts so the ACT ops can be deferred ---
            # part 1 (DVE): yb = py + b2; bn stats; rstd via bit-trick +
            # 1 Newton step; nmr = -mu*rstd.
            def ep_stats(py, nbc=NBC):
                yb = yp.tile([128, nbc, VOCAB], f32, tag="yb")
                nc.vector.tensor_add(yb[:], py[:], b2_t[:, 0:nbc, :])
                mvs = sp.tile([128, nbc, 2], f32, tag="mvs")
                stats = sp.tile([128, nbc, 6], f32, tag="stats")
                for bc in range(nbc):
                    nc.vector.bn_stats(stats[:, bc, :], yb[:, bc, :])
                for bc in range(nbc):
                    nc.vector.bn_aggr(mvs[:, bc, :], stats[:, bc, :])
                ve = sp.tile([128, nbc], f32, tag="ve")
                nc.vector.tensor_scalar(
                    out=ve[:], in0=mvs[:, :, 1], scalar1=LN_EPS, scalar2=None,
                    op0=mybir.AluOpType.add,
                )
                yi = sp.tile([128, nbc], mybir.dt.int32, tag="yi")
                nc.vector.tensor_scalar(
                    out=yi[:], in0=ve[:].bitcast(mybir.dt.int32), scalar1=1,
                    scalar2=None, op0=mybir.AluOpType.arith_shift_right,
                )
                rstd = sp.tile([128, nbc], f32, tag="rstd")
                nc.vector.tensor_sub(
                    rstd[:].bitcast(mybir.dt.int32), magic_t[:, 0:nbc], yi[:]
                )
                nt1 = sp.tile([128, nbc], f32, tag="nt1")
                nt2 = sp.tile([128, nbc], f32, tag="nt2")
                # 2 Newton steps: ~5e-6 rel err on rstd, below matmul noise
                for _ in range(2):
                    nc.vector.tensor_mul(nt1[:], rstd[:], rstd[:])
                    nc.vector.tensor_mul(nt2[:], nt1[:], ve[:])
                    nc.vector.tensor_scalar(
                        out=nt2[:], in0=nt2[:], scalar1=-0.5, scalar2=1.5,
                        op0=mybir.AluOpType.mult, op1=mybir.AluOpType.add,
                    )
                    nc.vector.tensor_mul(rstd[:], nt2[:], rstd[:])
                nmr = sp.tile([128, nbc], f32, tag="nmr")
                nc.vector.tensor_mul(nmr[:], mvs[:, :, 0], rstd[:])
                nc.vector.tensor_scalar(
                    out=nmr[:], in0=nmr[:], scalar1=-1.0, scalar2=None,
                    op0=mybir.AluOpType.mult,
                )
                return yb, mvs, rstd, nmr

            # part 2: gelu2 fused with the LN scale/shift (normal LN), then
            # skip-add on Pool and store. pool_add=False falls back to DVE.
            def ep_out(bs, c0, st, xb_t, pool_add=True, nbc=NBC):
                yb, mvs, rstd, nmr = st
                out_view = out[bs + c0 * 128 : bs + (c0 + nbc) * 128, :].rearrange(
                    "(c p) d -> p c d", p=128
                )
                g_t = op.tile([128, nbc, VOCAB], f32, tag="g")
                if general_ln or zbat:
                    # batched: z on DVE (per-bc scalars), ONE N=1024 gelu on
                    # ACT - minimizes ACT time at the cost of ~0.8us DVE
                    z_t = op.tile([128, nbc, VOCAB], f32, tag="z")
                    for bc in range(nbc):
                        nc.vector.tensor_scalar(
                            out=z_t[:, bc, :], in0=yb[:, bc, :],
                            scalar1=mvs[:, bc, 0:1], scalar2=rstd[:, bc : bc + 1],
                            op0=mybir.AluOpType.subtract, op1=mybir.AluOpType.mult,
                        )
                    if general_ln:
                        nc.vector.tensor_mul(z_t[:], z_t[:], lnw_t[:])
                        nc.vector.tensor_add(z_t[:], z_t[:], lnb_t[:])
                    nc.scalar.activation(g_t[:], z_t[:], Act.Gelu)
                else:
                    for bc in range(nbc):
                        nc.scalar.activation(
                            g_t[:, bc, :], yb[:, bc, :], Act.Gelu,
                            bias=nmr[:, bc : bc + 1], scale=rstd[:, bc : bc + 1],
                        )
                o_t = op.tile([128, nbc, VOCAB], bf16 if obf else f32, tag="o")
                adder = nc.gpsimd if pool_add else nc.vector
                adder.tensor_add(
                    o_t[:], g_t[:], xb_t[:, c0 : c0 + nbc, :]
                )
                nc.sync.dma_start(out_view[:], o_t[:])

            def g2_mms(h_prev, pys):
                # flat GEMM2 list for one superblock: 8 chunks, each one
                # complete (half, bc) accumulation group of 8 matmuls
                mms = []
                for hf in range(2):
                    for bc in range(NBC):
                        for ec in range(NEC):
                            mms.append(
                                lambda hf=hf, bc=bc, ec=ec: nc.tensor.matmul(
                                    pys[hf][:, bc, :],
                                    h_prev[
                                        :, ec,
                                        hf * 512 + bc * 128 : hf * 512 + (bc + 1) * 128,
                                    ],
                                    u2_t[:, ec, :],
                                    start=(ec == 0),
                                    stop=(ec == NEC - 1),
                                )
                            )
                return mms

            def g2_mms_q4(h_prev, pys):
                # quarter-granularity GEMM2: 8 chunks, each one complete
                # (quarter, bc2) accumulation group of 8 matmuls (1-bank py)
                mms = []
                for q in range(4):
                    for b2i in range(2):
                        for ec in range(NEC):
                            mms.append(
                                lambda q=q, b2i=b2i, ec=ec: nc.tensor.matmul(
                                    pys[q][:, b2i, :],
                                    h_prev[
                                        :, ec,
                                        q * 256 + b2i * 128 : q * 256 + (b2i + 1) * 128,
                                    ],
                                    u2_t[:, ec, :],
                                    start=(ec == 0),
                                    stop=(ec == NEC - 1),
                                )
                            )
                return mms

            g2_prev = None  # (bs, h_t, xb_t) of superblock i-1
            pend_b = None  # deferred ep_out for half B of superblock i-2
            pend_q = {}  # q4: deferred ep_out closures keyed by ec slot

            for it in range(NSB * reps):
                i = it % NSB
                bs = i * SBLK
                xt_t = xt_queue.pop(0)
                if it + xq <= NSB * reps - 1:
                    xt_queue.append(prefetch_xt(it + xq))

                if it == 1:
                    # b2 constants first needed by the first epilogue (during
                    # superblock 2) - emitted here so early xt prefetches win
                    # the DMA queue
                    nc.sync.dma_start(b2_t[:], b2bc[:])

                h_t = hp.tile([128, NEC, SBLK], g2dt)
                if g2_prev is not None:
                    bs_p, h_prev, xb_prev = g2_prev
                    if q4:
                        py_q0 = pyq.tile([128, 2, VOCAB], f32, tag="py")
                        py_q1 = pyq.tile([128, 2, VOCAB], f32, tag="py")
                        py_q2 = pyq.tile([128, 2, VOCAB], f32, tag="py")
                        py_q3 = pyq.tile([128, 2, VOCAB], f32, tag="py")
                        pys = [py_q0, py_q1, py_q2, py_q3]
                        g2 = g2_mms_q4(h_prev, pys)
                        stq = [None] * 4
                    else:
                        py_a = pyq.tile([128, NBC, VOCAB], f32, tag="py")
                        py_b = pyq.tile([128, NBC, VOCAB], f32, tag="py")
                        g2 = g2_mms(h_prev, (py_a, py_b))
                else:
                    py_a = py_b = g2 = None

                for ec in range(NEC):
                    ph = phq.tile([128, SBLK], f32, tag="ph")
                    # kc-outer so each u1 chunk is loaded once and streams
                    # both superblock halves (two open groups, distinct banks)
                    for kc in range(NKC):
                        for hf in range(2):
                            nc.tensor.matmul(
                                ph[:, hf * 512 : (hf + 1) * 512],
                                u1_t[:, kc, ec * 128 : (ec + 1) * 128],
                                xt_t[:, kc, hf * 512 : (hf + 1) * 512],
                                start=(kc == 0),
                                stop=(kc == NKC - 1),
                            )
                    nc.scalar.activation(
                        h_t[:, ec, :], ph[:], Act.Gelu, bias=b1_t[:, ec : ec + 1]
                    )
                    if g2 is not None:
                        for mm in g2[ec * NEC : (ec + 1) * NEC]:
                            mm()
                        if q4:
                            if ec % 2 == 1:
                                stq[ec // 2] = ep_stats(pys[ec // 2], nbc=2)
                        elif ec == 3:
                            st_a = ep_stats(py_a)
                        elif ec == 7:
                            st_b = ep_stats(py_b)
                    # gelu2/skip/store are spread into ACT's idle gaps between
                    # gelu1 ops instead of bunching at the superblock boundary
                    # (which would delay gelu1(i+1) and stall the PE on ph
                    # reuse): half B of superblock i-2 lands after ec2, half A
                    # of superblock i-1 after ec6 (its stats are ready ~ec5).
                    if q4:
                        if ec in pend_q:
                            pend_q.pop(ec)()
                        if g2 is not None:
                            if ec == 4:
                                ep_out(bs_p, 0, stq[0], xb_prev, nbc=2)
                            elif ec == 6:
                                ep_out(bs_p, 2, stq[1], xb_prev, nbc=2)
                    elif ec == 2 and pend_b is not None:
                        pend_b()
                        pend_b = None
                    elif ec == 6 and g2 is not None:
                        ep_out(bs_p, 0, st_a, xb_prev)

                # batch-major rows for the skip connection (needed by the
                # NEXT superblock's deferred ep_out)
                xb_t = xbp.tile([128, 2 * NBC, VOCAB], bf16 if xbbf else f32)
                nc.sync.dma_start(
                    xb_t[:], xb[bs : bs + SBLK, :].rearrange("(c p) d -> p c d", p=128)
                )

                if g2 is not None:
                    if q4:
                        pend_q = {
                            0: lambda b=bs_p, s=stq[2], xbv=xb_prev: ep_out(
                                b, 4, s, xbv, nbc=2
                            ),
                            2: lambda b=bs_p, s=stq[3], xbv=xb_prev: ep_out(
                                b, 6, s, xbv, nbc=2
                            ),
                        }
                    else:
                        pend_b = lambda b=bs_p, s=st_b, xbv=xb_prev: ep_out(
                            b, 4, s, xbv
                        )
                g2_prev = (bs, h_t, xb_t)

            # drain: GEMM2 of the last superblock, epilogues interleaved so
            # each chunk's DVE/ACT chain overlaps the remaining matmuls
            bs_p, h_prev, xb_prev = g2_prev
            if q4:
                py_q0 = pyq.tile([128, 2, VOCAB], f32, tag="py")
                py_q1 = pyq.tile([128, 2, VOCAB], f32, tag="py")
                py_q2 = pyq.tile([128, 2, VOCAB], f32, tag="py")
                py_q3 = pyq.tile([128, 2, VOCAB], f32, tag="py")
                pys = [py_q0, py_q1, py_q2, py_q3]
                g2 = g2_mms_q4(h_prev, pys)
                for slot in sorted(pend_q):
                    pend_q.pop(slot)()
                stq = [None] * 4
                for c in range(8):
                    for mm in g2[c * NEC : (c + 1) * NEC]:
                        mm()
                    if c % 2 == 1:
                        stq[c // 2] = ep_stats(pys[c // 2], nbc=2)
                        if c >= 3:
                            ep_out(bs_p, 2 * (c // 2 - 1), stq[c // 2 - 1], xb_prev, nbc=2)
                ep_out(bs_p, 4, stq[2], xb_prev, nbc=2)
                ep_out(bs_p, 6, stq[3], xb_prev, nbc=2)
            else:
                py_a = pyq.tile([128, NBC, VOCAB], f32, tag="py")
                py_b = pyq.tile([128, NBC, VOCAB], f32, tag="py")
                g2 = g2_mms(h_prev, (py_a, py_b))
                if pend_b is not None:
                    pend_b()
                for mm in g2[: 4 * NEC]:
                    mm()
                st_a = ep_stats(py_a)
                ep_out(bs_p, 0, st_a, xb_prev)
                for mm in g2[4 * NEC :]:
                    mm()
                st_b = ep_stats(py_b)
                ep_out(bs_p, 4, st_b, xb_prev)

    nc.compile()
    return nc


def _get_nc(general_ln: bool, reps: int = 1):
    key = ("nc", general_ln, reps)
    if key not in _CACHE:
        _CACHE[key] = _build(general_ln, reps)
    return _CACHE[key]


def make_in_map(x_n, U1_n, b1_n, U2_n, b2_n, ln_w=None, ln_b=None,
                g1bf=True, g2bf=True, xbbf=False, obf=False, zbat=False, q4=False, xq=3):
    """Per-branch input map. x_n: (B, D) f32; U1_n: (D, E); U2_n: (E, D)."""
    import ml_dtypes

    bf16 = ml_dtypes.bfloat16
    g1np = bf16 if g1bf else np.float32
    g2np = bf16 if g2bf else np.float32
    xb_n = np.ascontiguousarray(x_n, dtype=np.float32)
    m = {
        "xt": np.ascontiguousarray(xb_n.T.astype(g1np)),
        "xb": xb_n.astype(bf16) if xbbf else xb_n,
        "u1": np.ascontiguousarray(np.asarray(U1_n, np.float32).astype(g1np)),
        "u2": np.ascontiguousarray(np.asarray(U2_n, np.float32).astype(g2np)),
        "b1r": np.ascontiguousarray(np.asarray(b1_n, np.float32).reshape(NEC, 128).T),
        "b2bc": np.broadcast_to(np.asarray(b2_n, np.float32), (128, NBC, VOCAB)).copy(),
    }
    if ln_w is not None:
        m["lnwbc"] = np.broadcast_to(np.asarray(ln_w, np.float32), (128, NBC, VOCAB)).copy()
        m["lnbbc"] = np.broadcast_to(np.asarray(ln_b, np.float32), (128, NBC, VOCAB)).copy()
    return m


def kernel(x, U1, b1, U2, b2, ln_w, ln_b):
    global LAST_EXEC_NS
    from concourse.bass_utils import run_bass_kernel_spmd

    x = np.asarray(x, dtype=np.float32)
    U1 = np.asarray(U1, dtype=np.float32)
    b1 = np.asarray(b1, dtype=np.float32)
    U2 = np.asarray(U2, dtype=np.float32)
    b2 = np.asarray(b2, dtype=np.float32)
    ln_w = np.asarray(ln_w, dtype=np.float32)
    ln_b = np.asarray(ln_b, dtype=np.float32)

    general_ln = not (
        np.all(ln_w == np.float32(1.0)) and np.all(ln_b == np.float32(0.0))
    )
    nc = _get_nc(general_ln)

    in_maps = []
    for n in range(BRANCH):
        if general_ln:
            m = make_in_map(x[:, n, :], U1[n], b1[n], U2[n], b2[n], ln_w, ln_b)
        else:
            m = make_in_map(x[:, n, :], U1[n], b1[n], U2[n], b2[n])
        in_maps.append(m)

    res = run_bass_kernel_spmd(nc, in_maps, core_ids=list(range(BRANCH)))
    LAST_EXEC_NS = res.exec_time_ns

    outp = np.empty((BATCH, BRANCH, VOCAB), dtype=np.float32)
    for n in range(BRANCH):
        outp[:, n, :] = res.results[n]["out"]
    return outp.reshape(BATCH, 1, BRANCH * VOCAB)

